# revision 28
# baseline (speedup 1.0000x reference)
"""DiagGCN message-passing kernel for 8 Trainium2 NeuronCores (V2).

Strategy (receiver-sharded, no collectives, NO per-edge DMA descriptors):
  - Core c owns output rows [c*12500, (c+1)*12500). Edges are bucketed to
    cores by recv_idx, so each core computes its output slice completely.
  - The sender table is kept TRANSPOSED in SBUF ([128 d, rows]) in chunks
    of 12500 rows (8 chunks, double-buffered, 50 KB/partition each); the
    type table V_types^T [128, 401] is fully resident. Per span of 1024
    edge slots, gpsimd ap_gather pulls the per-edge sender/type columns
    out of SBUF — no HBM gather descriptors at all.
  - DVE multiplies the [d, e] tiles (fp16 out); ACT applies bias+relu in
    place; PE transposes each 128x128 tile ([d,e] -> [e,d], fp16, via
    identity) into a spare PSUM bank; ACT copies the transposed span to
    SBUF; DVE builds weighted one-hots oh[e, win] = (iota==recv)*w in
    fp16; PE contracts psum[d, win] += msg^T @ oh per tile (segment-sum
    as matmul).
  - Finished windows drain PSUM -> bf16 SBUF accumulator; one strided DMA
    (SP engine, like all bulk DMAs here) writes the [12500, 128] slice at
    the end. Host converts bf16 -> f32 and transposes.
"""
import sys
sys.path.insert(0, "/opt/trn_rl_repo")
import numpy as np
from dataclasses import dataclass


@dataclass(frozen=True)
class Config:
    n_nodes: int = 100000
    n_edges: int = 600000
    d: int = 128
    n_types: int = 401
    n_cores: int = 8
    chunks: int = 4          # sender-table chunks resident in SBUF
    span: int = 1024         # edge slots per gather call / compute span
    win: int = 512           # receiver window (matmul free dim)

    @property
    def npc(self):           # nodes per core
        return self.n_nodes // self.n_cores

    @property
    def nwin(self):          # windows per core
        return (self.npc + self.win - 1) // self.win

    @property
    def crows(self):         # sender rows per chunk
        return (self.n_nodes + self.chunks - 1) // self.chunks


CFG = Config()

_PROGRAM_CACHE = {}


def _wrap16(arr):
    """[NC, L] int -> [NC, 128, L/16] int16: idx j at [:, j%16, j//16], x8."""
    nc_, L = arr.shape
    a = arr.astype(np.int16).reshape(nc_, L // 16, 16).transpose(0, 2, 1)
    return np.ascontiguousarray(np.tile(a, (1, 8, 1)))


def _wrap128(arr, dtype=np.float32):
    """[NC, L] -> [NC, 128, L/128]: slot j at [:, j%128, j//128]."""
    nc_, L = arr.shape
    a = arr.astype(dtype).reshape(nc_, L // 128, 128).transpose(0, 2, 1)
    return np.ascontiguousarray(a)


def _schedule(S, cfg):
    """Static schedule from padded subgroup sizes S [chunks, nwin]."""
    chunks, nwin = S.shape
    offs = np.concatenate([[0], np.cumsum(S.ravel())])[:-1].reshape(chunks, nwin)
    L = int(S.sum())
    # window modes: first nonempty chunk copies, later chunks add
    first_chunk = np.full(nwin, -1, np.int64)
    for c in range(chunks):
        m = (S[c] > 0) & (first_chunk < 0)
        first_chunk[m] = c
    memset_windows = [w for w in range(nwin) if first_chunk[w] < 0]

    spans = []   # dict(off, n, chunk, tiles)
    gw = -1
    windows = []  # per nonempty (c,w): dict(c,w,gw,mode)
    for c in range(chunks):
        Lc = int(S[c].sum())
        if Lc == 0:
            continue
        c_off = int(offs[c, 0])
        tiles = []
        for w in range(nwin):
            nt = int(S[c, w]) // 128
            if nt == 0:
                continue
            gw += 1
            windows.append(dict(c=c, w=w, gw=gw,
                                mode="copy" if first_chunk[w] == c else "add"))
            for k in range(nt):
                tiles.append(dict(w=w, gw=gw, first=(k == 0), last=(k == nt - 1)))
        pos = 0
        while pos < Lc:
            n = min(cfg.span, Lc - pos)
            t0 = pos // 128
            spans.append(dict(off=c_off + pos, n=n, chunk=c,
                              tiles=tiles[t0:t0 + n // 128]))
            pos += n
    for sp in spans:
        sp["ending"] = [t["gw"] for t in sp["tiles"] if t["last"]]
    chunk_first_span = {}
    chunk_last_span = {}
    for s, sp in enumerate(spans):
        c = sp["chunk"]
        if c not in chunk_first_span:
            chunk_first_span[c] = s
        chunk_last_span[c] = s
    return dict(spans=spans, windows=windows, memset_windows=memset_windows,
                L=L, offs=offs, n_windows=gw + 1,
                chunk_first_span=chunk_first_span,
                chunk_last_span=chunk_last_span)


def _build_program(S_bytes, L, has_bias, cfg):
    import concourse.bacc as bacc
    import concourse.mybir as mybir
    from concourse.library_config import ap_gather as ap_gather_lib

    S = np.frombuffer(S_bytes, np.int64).reshape(cfg.chunks, cfg.nwin)
    sch = _schedule(S, cfg)
    spans, windows = sch["spans"], sch["windows"]
    nspan = len(spans)
    n_windows = sch["n_windows"]
    NWIN, D, WIN = cfg.nwin, cfg.d, cfg.win
    CROWS = cfg.crows
    f32 = mybir.dt.float32
    f16 = mybir.dt.float16
    bf16 = mybir.dt.bfloat16
    chunks_used = sorted(sch["chunk_first_span"].keys())
    last_span_of = sch["chunk_last_span"]

    nc = bacc.Bacc("TRN2", debug=True)
    # DRAM inputs. vtabt is host-transposed: [d, node] so chunk slices are
    # strided row reads.
    vtabt = nc.dram_tensor("vtabt", [D, cfg.n_nodes], f32, kind="ExternalInput")
    vtypt = nc.dram_tensor("vtypt", [D, cfg.n_types], f32, kind="ExternalInput")
    sidx_d = nc.dram_tensor("sidx", [128, L // 16], mybir.dt.int16, kind="ExternalInput")
    tidx_d = nc.dram_tensor("tidx", [128, L // 16], mybir.dt.int16, kind="ExternalInput")
    recvf_d = nc.dram_tensor("recvf", [128, L // 128], f32, kind="ExternalInput")
    wf_d = nc.dram_tensor("wf", [128, L // 128], f32, kind="ExternalInput")
    iota_d = nc.dram_tensor("iota", [128, WIN], f16, kind="ExternalInput")
    ident_d = nc.dram_tensor("ident", [128, 128], f16, kind="ExternalInput")
    if has_bias:
        bias_d = nc.dram_tensor("biasd", [128, 1], f32, kind="ExternalInput")
    out_d = nc.dram_tensor("out", [128, NWIN * WIN], f16, kind="ExternalOutput")

    from contextlib import ExitStack
    with ExitStack() as ctx:
        tb = [ctx.enter_context(nc.sbuf_tensor("tb0", [128, CROWS], f32))]
        typt = ctx.enter_context(nc.sbuf_tensor("typt", [128, cfg.n_types], f32))
        sidx_t = ctx.enter_context(nc.sbuf_tensor("sidx_t", [128, L // 16], mybir.dt.int16))
        tidx_t = ctx.enter_context(nc.sbuf_tensor("tidx_t", [128, L // 16], mybir.dt.int16))
        recvf_t = ctx.enter_context(nc.sbuf_tensor("recvf_t", [128, L // 128], f32))
        wf_t = ctx.enter_context(nc.sbuf_tensor("wf_t", [128, L // 128], f32))
        iota_t = ctx.enter_context(nc.sbuf_tensor("iota_t", [128, WIN], f16))
        ident_t = ctx.enter_context(nc.sbuf_tensor("ident_t", [128, 128], f16))
        if has_bias:
            bias_t = ctx.enter_context(nc.sbuf_tensor("bias_t", [128, 1], f32))
        NG = 2            # gather span buffers (sgT/tgT)
        sgT = [ctx.enter_context(nc.sbuf_tensor(f"sgT{i}", [128, cfg.span], f32))
               for i in range(NG)]
        tgT = [ctx.enter_context(nc.sbuf_tensor(f"tgT{i}", [128, cfg.span], f32))
               for i in range(NG)]
        NM = 3            # mul/relu output (PE transpose input)
        mrel = [ctx.enter_context(nc.sbuf_tensor(f"mrel{i}", [128, cfg.span], f16))
                for i in range(NM)]
        NR = 3            # transposed span + one-hots
        rspan = [ctx.enter_context(nc.sbuf_tensor(f"rspan{i}", [128, cfg.span], f16))
                 for i in range(NR)]
        SPT = cfg.span // 128
        NO = 2
        ohb = [ctx.enter_context(nc.sbuf_tensor(f"ohb{i}", [128, SPT, WIN], f16))
               for i in range(NO)]
        accum = ctx.enter_context(nc.sbuf_tensor("accum", [128, NWIN * WIN], f16))
        psum = ctx.enter_context(nc.psum_tensor("psum", [128, 8, 512], f32))

        ld = ctx.enter_context(nc.semaphore("ld"))     # aux loads (SP dma)
        ldc = ctx.enter_context(nc.semaphore("ldc"))   # type+chunk table loads
        gs = ctx.enter_context(nc.semaphore("gs"))     # gathers done (pool)
        vm = ctx.enter_context(nc.semaphore("vm"))     # DVE mul done
        ar = ctx.enter_context(nc.semaphore("ar"))     # ACT relu done
        pt = ctx.enter_context(nc.semaphore("pt"))     # PE transposes done
        ac = ctx.enter_context(nc.semaphore("ac"))     # ACT psum copy done
        ohs = ctx.enter_context(nc.semaphore("ohs"))   # one-hot built
        mm = ctx.enter_context(nc.semaphore("mm"))     # window matmul done
        rs = ctx.enter_context(nc.semaphore("rs"))     # window drained
        pes = ctx.enter_context(nc.semaphore("pes"))   # PE scatter span done
        od = ctx.enter_context(nc.semaphore("od"))     # output dma
        fin = ctx.enter_context(nc.semaphore("fin"))
        block = ctx.enter_context(nc.Block())

        n_aux = 7 if has_bias else 6

        # ---- SP: all bulk DMA ----
        @block.sync
        def _(sp):
            sp.dma_start(sidx_t[:], sidx_d[:]).then_inc(ld, 16)
            sp.dma_start(tidx_t[:], tidx_d[:]).then_inc(ld, 16)
            sp.dma_start(recvf_t[:], recvf_d[:]).then_inc(ld, 16)
            sp.dma_start(wf_t[:], wf_d[:]).then_inc(ld, 16)
            sp.dma_start(iota_t[:], iota_d[:]).then_inc(ld, 16)
            sp.dma_start(ident_t[:], ident_d[:]).then_inc(ld, 16)
            if has_bias:
                sp.dma_start(bias_t[:], bias_d[:]).then_inc(ld, 16)
            sp.dma_start(typt[:], vtypt[:]).then_inc(ldc, 16)
            # chunk table loads (single buffer): load chunk i after pool
            # finished gathering from chunk i-1.
            for i, c in enumerate(chunks_used):
                if i >= 1:
                    prev_last = last_span_of[chunks_used[i - 1]]
                    sp.wait_ge(gs, 2 * (prev_last + 1))
                c0 = c * CROWS
                crn = min(CROWS, cfg.n_nodes - c0)
                sp.dma_start(tb[0][:, :crn], vtabt[:, c0:c0 + crn]
                             ).then_inc(ldc, 16)
            # final output store (host transposes [128, nodes] -> [nodes,128])
            sp.wait_ge(rs, n_windows)
            sp.wait_ge(fin, 1)
            sp.dma_start(out_d[:], accum[:]).then_inc(od, 16)
            sp.wait_ge(od, 16)

        # ---- Pool: on-chip gathers ----
        @block.gpsimd
        def _(g):
            g.load_library(ap_gather_lib)
            g.wait_ge(ld, 32)   # sidx+tidx loaded
            for s, sp_ in enumerate(spans):
                k, n, off = s % NG, sp_["n"], sp_["off"]
                ci = chunks_used.index(sp_["chunk"])
                g.wait_ge(ldc, 16 * (ci + 2))   # typt + chunk tables ready
                if s >= NG:
                    g.wait_ge(vm, s - NG + 1)   # sgT/tgT buffer free
                crn = min(CROWS, cfg.n_nodes - sp_["chunk"] * CROWS)
                g.ap_gather(sgT[k][:, :n], tb[0][:, :crn],
                            sidx_t[:, off // 16:(off + n) // 16],
                            128, crn, 1, n).then_inc(gs, 1)
                g.ap_gather(tgT[k][:, :n], typt[:, :],
                            tidx_t[:, off // 16:(off + n) // 16],
                            128, cfg.n_types, 1, n).then_inc(gs, 1)

        # ---- DVE: one-hots, window drains, message multiply ----
        @block.vector
        def _(v):
            v.wait_ge(ld, 16 * n_aux)
            for w in sch["memset_windows"]:
                v.memset(accum[:, w * WIN:(w + 1) * WIN], 0.0)

            def drain_window(wi):
                win = windows[wi]
                v.wait_ge(mm, wi + 1)
                dst = accum[:, win["w"] * WIN:(win["w"] + 1) * WIN]
                src = psum[:, wi % 6, :]
                if win["mode"] == "copy":
                    v.tensor_copy(dst, src)
                else:
                    v.tensor_add(dst, dst, src)
                v.sem_inc(rs, 1)

            for s, sp_ in enumerate(spans):
                k, n, off = s % NG, sp_["n"], sp_["off"]
                ko = s % NO
                nt = n // 128
                # one-hots first: only need preloaded recvf/wf/iota + buffer
                if s >= NO:
                    v.wait_ge(pes, s - NO + 1)
                for i in range(nt):
                    col = off // 128 + i
                    v.tensor_scalar(
                        ohb[ko][:, i, :],
                        iota_t[:, :].rearrange("p (o d) -> p o d", o=1),
                        recvf_t[:, col:col + 1],
                        wf_t[:, col:col + 1],
                        mybir.AluOpType.is_equal,
                        mybir.AluOpType.mult,
                    )
                v.drain().then_inc(ohs, 1)
                # drain windows finished two spans back (lag-2: S(s-2) done)
                if s >= 2:
                    for wi in spans[s - 2]["ending"]:
                        drain_window(wi)
                # message multiply (fp16 out)
                v.wait_ge(gs, 2 * (s + 1))
                if s >= NM:
                    v.wait_ge(pt, s - NM + 1)   # mrel buffer free
                v.tensor_mul(mrel[s % NM][:, :n], sgT[k][:, :n], tgT[k][:, :n])
                v.drain().then_inc(vm, 1)
            for sp_ in spans[-2:]:
                for wi in sp_["ending"]:
                    drain_window(wi)
            v.drain().then_inc(fin, 1)

        # ---- ACT: bias+relu (in [d, e]); PSUM->SBUF copy of transposes ----
        @block.scalar
        def _(a):
            if has_bias:
                a.wait_ge(ld, 16 * n_aux)

            def copy_span(j):
                n = spans[j]["n"]
                if j >= NR:
                    a.wait_ge(pes, j - NR + 1)  # rspan buffer free
                a.wait_ge(pt, j + 1)
                a.activation(rspan[j % NR][:, :n],
                             psum[:, 6 + j % 2, :n // 2].bitcast(f16),
                             mybir.ActivationFunctionType.Copy)
                a.drain().then_inc(ac, 1)

            for s, sp_ in enumerate(spans):
                n = sp_["n"]
                km = s % NM
                a.wait_ge(vm, s + 1)
                a.activation(mrel[km][:, :n], mrel[km][:, :n],
                             mybir.ActivationFunctionType.Relu,
                             bias=bias_t[:, :] if has_bias else 0.0)
                a.drain().then_inc(ar, 1)
                if s >= 1:
                    copy_span(s - 1)
            copy_span(nspan - 1)

        # ---- PE: per-tile transposes, then scatter matmuls (1-span lag) ----
        @block.tensor
        def _(t):
            t.wait_ge(ld, 16 * 6)   # ident loaded

            def scatter_span(j):
                kr, ko = j % NR, j % NO
                t.wait_ge(ac, j + 1)
                t.wait_ge(ohs, j + 1)
                for i, tile in enumerate(spans[j]["tiles"]):
                    gw = tile["gw"]
                    if tile["first"] and gw >= 6:
                        t.wait_ge(rs, gw - 5)
                    inst = t.matmul(
                        psum[:, gw % 6, :],
                        rspan[kr][:, i * 128:(i + 1) * 128],
                        ohb[ko][:, i, :],
                        start=tile["first"], stop=tile["last"],
                    )
                    if tile["last"]:
                        inst.then_inc(mm, 1)
                t.drain().then_inc(pes, 1)

            for s, sp_ in enumerate(spans):
                n = sp_["n"]
                km = s % NM
                nt = n // 128
                t.wait_ge(ar, s + 1)
                if s >= 2:
                    t.wait_ge(ac, s - 1)  # previous span in this bank copied
                pb = psum[:, 6 + s % 2, :n // 2].bitcast(f16)
                for i in range(nt):
                    t.transpose(pb[:, i * 128:(i + 1) * 128],
                                mrel[km][:, i * 128:(i + 1) * 128],
                                ident_t[:, :])
                t.drain().then_inc(pt, 1)
                if s >= 1:
                    scatter_span(s - 1)
            scatter_span(nspan - 1)

    nc.compile()
    return nc


def _get_program(S, L, has_bias, cfg):
    key = (S.tobytes(), L, has_bias, cfg)
    if key not in _PROGRAM_CACHE:
        _PROGRAM_CACHE[key] = _build_program(S.tobytes(), L, has_bias, cfg)
    return _PROGRAM_CACHE[key]


def _prepare(V, VT, B, w, snd, typ, rcv, cfg):
    NC, NPC, NWIN, CH = cfg.n_cores, cfg.npc, cfg.nwin, cfg.chunks
    E = cfg.n_edges
    snd = np.asarray(snd, np.int64)
    typ = np.asarray(typ, np.int64)
    rcv = np.asarray(rcv, np.int64)
    w = np.asarray(w, np.float32)

    core = rcv // NPC
    rloc = rcv - core * NPC
    win = rloc // cfg.win
    rin = (rloc - win * cfg.win).astype(np.float32)
    chunk = snd // cfg.crows
    sloc = (snd - chunk * cfg.crows).astype(np.int16)

    key = (core * CH + chunk) * NWIN + win
    order = np.lexsort((sloc, key))
    cnt = np.bincount(key, minlength=NC * CH * NWIN).reshape(NC, CH, NWIN)
    S = ((cnt.max(axis=0) + 127) // 128 * 128).astype(np.int64)  # [CH, NWIN]
    offs = np.concatenate([[0], np.cumsum(S.ravel())])[:-1].reshape(CH, NWIN)
    L = int(S.sum())

    cnt_flat = cnt.ravel()
    grp_start = np.concatenate([[0], np.cumsum(cnt_flat)])[:-1]
    ranks = np.arange(E) - np.repeat(grp_start, cnt_flat)
    ids = key[order]
    c_of = (ids // NWIN) % CH
    w_of = ids % NWIN
    slot = offs[c_of, w_of] + ranks
    core_s = core[order]

    sl_s = np.zeros((NC, L), np.int16)
    sl_t = np.zeros((NC, L), np.int16)
    sl_r = np.zeros((NC, L), np.float32)
    sl_w = np.zeros((NC, L), np.float32)
    sl_s[core_s, slot] = sloc[order]
    sl_t[core_s, slot] = typ[order].astype(np.int16)
    sl_r[core_s, slot] = rin[order]
    sl_w[core_s, slot] = w[order]

    iota = np.ascontiguousarray(
        np.tile(np.arange(cfg.win, dtype=np.float16), (128, 1)))
    ident = np.eye(128, dtype=np.float16)
    has_bias = bool(np.any(np.asarray(B) != 0))
    sidx_w = _wrap16(sl_s)
    tidx_w = _wrap16(sl_t)
    recvf = _wrap128(sl_r)
    wf = _wrap128(sl_w)
    VTt = np.ascontiguousarray(np.asarray(V, np.float32).T)       # [128, N]
    VTypt = np.ascontiguousarray(np.asarray(VT, np.float32).T)    # [128, 401]
    in_maps = []
    for i in range(NC):
        m = {"vtabt": VTt, "vtypt": VTypt, "sidx": sidx_w[i],
             "tidx": tidx_w[i], "recvf": recvf[i], "wf": wf[i],
             "iota": iota, "ident": ident}
        if has_bias:
            m["biasd"] = np.ascontiguousarray(
                np.asarray(B, np.float32).reshape(128, 1))
        in_maps.append(m)
    return S, L, has_bias, in_maps


def _run(V, VT, B, w, snd, typ, rcv, cfg=None, trace=False):
    from concourse.bass_utils import run_bass_kernel_spmd
    cfg = cfg or CFG
    S, L, has_bias, in_maps = _prepare(V, VT, B, w, snd, typ, rcv, cfg)
    nc = _get_program(S, L, has_bias, cfg)
    res = run_bass_kernel_spmd(nc, in_maps, list(range(cfg.n_cores)),
                               trace=trace)
    out = np.concatenate(
        [np.asarray(res.results[i]["out"]).astype(np.float32)[:, :cfg.npc].T
         for i in range(cfg.n_cores)], 0)
    return np.ascontiguousarray(out[:cfg.n_nodes]), res


def kernel(V_proj_sender, V_types, B_message, inc_weights,
           sender_idx, type_idx, recv_idx):
    out, _ = _run(V_proj_sender, V_types, B_message, inc_weights,
                  sender_idx, type_idx, recv_idx)
    return out


# revision 41
# speedup vs baseline: 1.0322x; 1.0322x over previous
"""DiagGCN message-passing kernel for 8 Trainium2 NeuronCores (V2).

Strategy (receiver-sharded, no collectives, NO per-edge DMA descriptors):
  - Core c owns output rows [c*12500, (c+1)*12500). Edges are bucketed to
    cores by recv_idx, so each core computes its output slice completely.
  - The sender table is kept TRANSPOSED in SBUF ([128 d, rows]) in chunks
    of 12500 rows (8 chunks, double-buffered, 50 KB/partition each); the
    type table V_types^T [128, 401] is fully resident. Per span of 1024
    edge slots, gpsimd ap_gather pulls the per-edge sender/type columns
    out of SBUF — no HBM gather descriptors at all.
  - DVE multiplies the [d, e] tiles (fp16 out); ACT applies bias+relu in
    place; PE transposes each 128x128 tile ([d,e] -> [e,d], fp16, via
    identity) into a spare PSUM bank; ACT copies the transposed span to
    SBUF; DVE builds weighted one-hots oh[e, win] = (iota==recv)*w in
    fp16; PE contracts psum[d, win] += msg^T @ oh per tile (segment-sum
    as matmul).
  - Finished windows drain PSUM -> bf16 SBUF accumulator; one strided DMA
    (SP engine, like all bulk DMAs here) writes the [12500, 128] slice at
    the end. Host converts bf16 -> f32 and transposes.
"""
import sys
sys.path.insert(0, "/opt/trn_rl_repo")
import numpy as np
from dataclasses import dataclass


@dataclass(frozen=True)
class Config:
    n_nodes: int = 100000
    n_edges: int = 600000
    d: int = 128
    n_types: int = 401
    n_cores: int = 8
    chunks: int = 4          # sender-table chunks resident in SBUF
    span: int = 1024         # edge slots per gather call / compute span
    win: int = 1024          # receiver window (2 PSUM banks per window)

    @property
    def npc(self):           # nodes per core
        return self.n_nodes // self.n_cores

    @property
    def nwin(self):          # windows per core
        return (self.npc + self.win - 1) // self.win

    @property
    def crows(self):         # sender rows per chunk
        return (self.n_nodes + self.chunks - 1) // self.chunks


CFG = Config()

_PROGRAM_CACHE = {}


def _wrap16(arr):
    """[NC, L] int -> [NC, 128, L/16] int16: idx j at [:, j%16, j//16], x8."""
    nc_, L = arr.shape
    a = arr.astype(np.int16).reshape(nc_, L // 16, 16).transpose(0, 2, 1)
    return np.ascontiguousarray(np.tile(a, (1, 8, 1)))


def _wrap128(arr, dtype=np.float32):
    """[NC, L] -> [NC, 128, L/128]: slot j at [:, j%128, j//128]."""
    nc_, L = arr.shape
    a = arr.astype(dtype).reshape(nc_, L // 128, 128).transpose(0, 2, 1)
    return np.ascontiguousarray(a)


def _schedule(S, cfg):
    """Static schedule from padded subgroup sizes S [chunks, nwin]."""
    chunks, nwin = S.shape
    offs = np.concatenate([[0], np.cumsum(S.ravel())])[:-1].reshape(chunks, nwin)
    L = int(S.sum())
    # window modes: first nonempty chunk copies, later chunks add
    first_chunk = np.full(nwin, -1, np.int64)
    for c in range(chunks):
        m = (S[c] > 0) & (first_chunk < 0)
        first_chunk[m] = c
    memset_windows = [w for w in range(nwin) if first_chunk[w] < 0]

    spans = []   # dict(off, n, chunk, tiles)
    gw = -1
    windows = []  # per nonempty (c,w): dict(c,w,gw,mode)
    for c in range(chunks):
        Lc = int(S[c].sum())
        if Lc == 0:
            continue
        c_off = int(offs[c, 0])
        tiles = []
        for w in range(nwin):
            nt = int(S[c, w]) // 128
            if nt == 0:
                continue
            gw += 1
            windows.append(dict(c=c, w=w, gw=gw,
                                mode="copy" if first_chunk[w] == c else "add"))
            for k in range(nt):
                tiles.append(dict(w=w, gw=gw, first=(k == 0), last=(k == nt - 1)))
        pos = 0
        while pos < Lc:
            n = min(cfg.span, Lc - pos)
            t0 = pos // 128
            spans.append(dict(off=c_off + pos, n=n, chunk=c,
                              tiles=tiles[t0:t0 + n // 128]))
            pos += n
    for sp in spans:
        sp["ending"] = [t["gw"] for t in sp["tiles"] if t["last"]]
    chunk_first_span = {}
    chunk_last_span = {}
    for s, sp in enumerate(spans):
        c = sp["chunk"]
        if c not in chunk_first_span:
            chunk_first_span[c] = s
        chunk_last_span[c] = s
    return dict(spans=spans, windows=windows, memset_windows=memset_windows,
                L=L, offs=offs, n_windows=gw + 1,
                chunk_first_span=chunk_first_span,
                chunk_last_span=chunk_last_span)


def _build_program(S_bytes, L, has_bias, cfg):
    import concourse.bacc as bacc
    import concourse.mybir as mybir
    from concourse.library_config import ap_gather as ap_gather_lib

    S = np.frombuffer(S_bytes, np.int64).reshape(cfg.chunks, cfg.nwin)
    sch = _schedule(S, cfg)
    spans, windows = sch["spans"], sch["windows"]
    nspan = len(spans)
    n_windows = sch["n_windows"]
    NWIN, D, WIN = cfg.nwin, cfg.d, cfg.win
    CROWS = cfg.crows
    f32 = mybir.dt.float32
    f16 = mybir.dt.float16
    bf16 = mybir.dt.bfloat16
    chunks_used = sorted(sch["chunk_first_span"].keys())
    last_span_of = sch["chunk_last_span"]

    nc = bacc.Bacc("TRN2", debug=True)
    # DRAM inputs. vtabt is host-transposed: [d, node] so chunk slices are
    # strided row reads.
    vtabt = nc.dram_tensor("vtabt", [D, cfg.n_nodes], f32, kind="ExternalInput")
    vtypt = nc.dram_tensor("vtypt", [D, cfg.n_types], f32, kind="ExternalInput")
    sidx_d = nc.dram_tensor("sidx", [128, L // 16], mybir.dt.int16, kind="ExternalInput")
    tidx_d = nc.dram_tensor("tidx", [128, L // 16], mybir.dt.int16, kind="ExternalInput")
    recvf_d = nc.dram_tensor("recvf", [128, L // 128], f32, kind="ExternalInput")
    wf_d = nc.dram_tensor("wf", [128, L // 128], f32, kind="ExternalInput")
    iota_d = nc.dram_tensor("iota", [128, WIN], f16, kind="ExternalInput")
    ident_d = nc.dram_tensor("ident", [128, 128], f16, kind="ExternalInput")
    if has_bias:
        bias_d = nc.dram_tensor("biasd", [128, 1], f32, kind="ExternalInput")
    out_d = nc.dram_tensor("out", [128, NWIN * WIN], f16, kind="ExternalOutput")

    from contextlib import ExitStack
    with ExitStack() as ctx:
        tb = [ctx.enter_context(nc.sbuf_tensor("tb0", [128, CROWS], f32))]
        typt = ctx.enter_context(nc.sbuf_tensor("typt", [128, cfg.n_types], f32))
        sidx_t = ctx.enter_context(nc.sbuf_tensor("sidx_t", [128, L // 16], mybir.dt.int16))
        tidx_t = ctx.enter_context(nc.sbuf_tensor("tidx_t", [128, L // 16], mybir.dt.int16))
        recvf_t = ctx.enter_context(nc.sbuf_tensor("recvf_t", [128, L // 128], f32))
        wf_t = ctx.enter_context(nc.sbuf_tensor("wf_t", [128, L // 128], f32))
        iota_t = ctx.enter_context(nc.sbuf_tensor("iota_t", [128, WIN], f16))
        ident_t = ctx.enter_context(nc.sbuf_tensor("ident_t", [128, 128], f16))
        if has_bias:
            bias_t = ctx.enter_context(nc.sbuf_tensor("bias_t", [128, 1], f32))
        sgT = [ctx.enter_context(nc.sbuf_tensor(f"sgT{i}", [128, cfg.span], f32))
               for i in range(2)]
        tgT = [ctx.enter_context(nc.sbuf_tensor("tgT0", [128, cfg.span], f32))]
        NM = 2            # mul/relu output (PE transpose input)
        mrel = [ctx.enter_context(nc.sbuf_tensor(f"mrel{i}", [128, cfg.span], f16))
                for i in range(NM)]
        NR = 2            # transposed span buffers
        rspan = [ctx.enter_context(nc.sbuf_tensor(f"rspan{i}", [128, cfg.span], f16))
                 for i in range(NR)]
        SPT = cfg.span // 128
        NO = 2
        ohb = [ctx.enter_context(nc.sbuf_tensor(f"ohb{i}", [128, SPT, WIN], f16))
               for i in range(NO)]
        accum = ctx.enter_context(nc.sbuf_tensor("accum", [128, NWIN * WIN], f16))
        psum = ctx.enter_context(nc.psum_tensor("psum", [128, 8, 512], f32))

        ld = ctx.enter_context(nc.semaphore("ld"))     # aux loads (SP dma)
        ldc = ctx.enter_context(nc.semaphore("ldc"))   # type+chunk table loads
        gs = ctx.enter_context(nc.semaphore("gs"))     # gathers done (pool)
        vm = ctx.enter_context(nc.semaphore("vm"))     # DVE mul done
        ar = ctx.enter_context(nc.semaphore("ar"))     # ACT relu done
        pt = ctx.enter_context(nc.semaphore("pt"))     # PE transposes done
        ac = ctx.enter_context(nc.semaphore("ac"))     # ACT psum copy done
        ohs = ctx.enter_context(nc.semaphore("ohs"))   # one-hot built
        mm = ctx.enter_context(nc.semaphore("mm"))     # window matmul done
        rs = ctx.enter_context(nc.semaphore("rs"))     # window drained
        pes = ctx.enter_context(nc.semaphore("pes"))   # PE scatter span done
        od = ctx.enter_context(nc.semaphore("od"))     # output dma
        fin = ctx.enter_context(nc.semaphore("fin"))
        block = ctx.enter_context(nc.Block())

        n_aux = 7 if has_bias else 6

        # ---- SP: all bulk DMA ----
        @block.sync
        def _(sp):
            sp.dma_start(sidx_t[:], sidx_d[:]).then_inc(ld, 16)
            sp.dma_start(tidx_t[:], tidx_d[:]).then_inc(ld, 16)
            sp.dma_start(recvf_t[:], recvf_d[:]).then_inc(ld, 16)
            sp.dma_start(wf_t[:], wf_d[:]).then_inc(ld, 16)
            sp.dma_start(iota_t[:], iota_d[:]).then_inc(ld, 16)
            sp.dma_start(ident_t[:], ident_d[:]).then_inc(ld, 16)
            if has_bias:
                sp.dma_start(bias_t[:], bias_d[:]).then_inc(ld, 16)
            sp.dma_start(typt[:], vtypt[:]).then_inc(ldc, 16)
            # chunk table loads (single buffer): load chunk i after pool
            # finished gathering from chunk i-1.
            for i, c in enumerate(chunks_used):
                if i >= 1:
                    prev_last = last_span_of[chunks_used[i - 1]]
                    sp.wait_ge(gs, 2 * (prev_last + 1))
                c0 = c * CROWS
                crn = min(CROWS, cfg.n_nodes - c0)
                sp.dma_start(tb[0][:, :crn], vtabt[:, c0:c0 + crn]
                             ).then_inc(ldc, 16)
            # final output store (host transposes [128, nodes] -> [nodes,128])
            sp.wait_ge(rs, n_windows)
            sp.wait_ge(fin, 1)
            sp.dma_start(out_d[:], accum[:]).then_inc(od, 16)
            sp.wait_ge(od, 16)

        # ---- Pool: on-chip gathers ----
        @block.gpsimd
        def _(g):
            g.load_library(ap_gather_lib)
            g.wait_ge(ld, 32)   # sidx+tidx loaded
            for s, sp_ in enumerate(spans):
                k, n, off = s % 2, sp_["n"], sp_["off"]
                ci = chunks_used.index(sp_["chunk"])
                g.wait_ge(ldc, 16 * (ci + 2))   # typt + chunk tables ready
                if s >= 2:
                    g.wait_ge(vm, s - 1)        # sgT[k] free (mul s-2 done)
                crn = min(CROWS, cfg.n_nodes - sp_["chunk"] * CROWS)
                g.ap_gather(sgT[k][:, :n], tb[0][:, :crn],
                            sidx_t[:, off // 16:(off + n) // 16],
                            128, crn, 1, n).then_inc(gs, 1)
                if s >= 1:
                    g.wait_ge(vm, s)            # tgT free (mul s-1 done)
                g.ap_gather(tgT[0][:, :n], typt[:, :],
                            tidx_t[:, off // 16:(off + n) // 16],
                            128, cfg.n_types, 1, n).then_inc(gs, 1)

        # ---- DVE: one-hots, window drains, message multiply ----
        @block.vector
        def _(v):
            v.wait_ge(ld, 16 * n_aux)
            for w in sch["memset_windows"]:
                v.memset(accum[:, w * WIN:(w + 1) * WIN], 0.0)

            def drain_window(wi):
                win = windows[wi]
                v.wait_ge(mm, wi + 1)
                dst = accum[:, win["w"] * WIN:(win["w"] + 1) * WIN]
                b = 2 * (wi % 3)
                src = psum[:, b:b + 2, :].rearrange("p a b -> p (a b)")
                if win["mode"] == "copy":
                    v.tensor_copy(dst, src)
                else:
                    v.tensor_add(dst, dst, src)
                v.sem_inc(rs, 1)

            for s, sp_ in enumerate(spans):
                k, n, off = s % 2, sp_["n"], sp_["off"]
                ko = s % NO
                nt = n // 128
                # one-hots first: only need preloaded recvf/wf/iota + buffer
                if s >= NO:
                    v.wait_ge(pes, s - NO + 1)
                for i in range(nt):
                    col = off // 128 + i
                    v.tensor_scalar(
                        ohb[ko][:, i, :],
                        iota_t[:, :].rearrange("p (o d) -> p o d", o=1),
                        recvf_t[:, col:col + 1],
                        wf_t[:, col:col + 1],
                        mybir.AluOpType.is_equal,
                        mybir.AluOpType.mult,
                    )
                v.drain().then_inc(ohs, 1)
                # drain windows finished two spans back (lag-2: S(s-2) done)
                if s >= 2:
                    for wi in spans[s - 2]["ending"]:
                        drain_window(wi)
                # message multiply (fp16 out)
                v.wait_ge(gs, 2 * (s + 1))
                if s >= NM:
                    v.wait_ge(pt, s - NM + 1)   # mrel buffer free
                v.tensor_mul(mrel[s % NM][:, :n], sgT[k][:, :n], tgT[0][:, :n])
                v.drain().then_inc(vm, 1)
            for sp_ in spans[-2:]:
                for wi in sp_["ending"]:
                    drain_window(wi)
            v.drain().then_inc(fin, 1)

        # ---- ACT: bias+relu (in [d, e]); PSUM->SBUF copy of transposes ----
        @block.scalar
        def _(a):
            if has_bias:
                a.wait_ge(ld, 16 * n_aux)

            def copy_span(j):
                n = spans[j]["n"]
                if j >= NR:
                    a.wait_ge(pes, j - NR + 1)  # rspan buffer free
                a.wait_ge(pt, j + 1)
                a.activation(rspan[j % NR][:, :n],
                             psum[:, 6 + j % 2, :n // 2].bitcast(f16),
                             mybir.ActivationFunctionType.Copy)
                a.drain().then_inc(ac, 1)

            for s, sp_ in enumerate(spans):
                n = sp_["n"]
                km = s % NM
                a.wait_ge(vm, s + 1)
                a.activation(mrel[km][:, :n], mrel[km][:, :n],
                             mybir.ActivationFunctionType.Relu,
                             bias=bias_t[:, :] if has_bias else 0.0)
                a.drain().then_inc(ar, 1)
                if s >= 1:
                    copy_span(s - 1)
            copy_span(nspan - 1)

        # ---- PE: per-tile transposes, then scatter matmuls (1-span lag) ----
        @block.tensor
        def _(t):
            t.wait_ge(ld, 16 * 6)   # ident loaded

            def scatter_span(j):
                kr, ko = j % NR, j % NO
                t.wait_ge(ac, j + 1)
                t.wait_ge(ohs, j + 1)
                for i, tile in enumerate(spans[j]["tiles"]):
                    gw = tile["gw"]
                    if tile["first"] and gw >= 3:
                        t.wait_ge(rs, gw - 2)
                    b = 2 * (gw % 3)
                    for h in range(2):
                        inst = t.matmul(
                            psum[:, b + h, :],
                            rspan[kr][:, i * 128:(i + 1) * 128],
                            ohb[ko][:, i, h * 512:(h + 1) * 512],
                            start=tile["first"], stop=tile["last"],
                        )
                    if tile["last"]:
                        inst.then_inc(mm, 1)
                t.drain().then_inc(pes, 1)

            for s, sp_ in enumerate(spans):
                n = sp_["n"]
                km = s % NM
                nt = n // 128
                t.wait_ge(ar, s + 1)
                if s >= 2:
                    t.wait_ge(ac, s - 1)  # previous span in this bank copied
                pb = psum[:, 6 + s % 2, :n // 2].bitcast(f16)
                for i in range(nt):
                    t.transpose(pb[:, i * 128:(i + 1) * 128],
                                mrel[km][:, i * 128:(i + 1) * 128],
                                ident_t[:, :])
                t.drain().then_inc(pt, 1)
                if s >= 1:
                    scatter_span(s - 1)
            scatter_span(nspan - 1)

    nc.compile()
    return nc


def _get_program(S, L, has_bias, cfg):
    key = (S.tobytes(), L, has_bias, cfg)
    if key not in _PROGRAM_CACHE:
        _PROGRAM_CACHE[key] = _build_program(S.tobytes(), L, has_bias, cfg)
    return _PROGRAM_CACHE[key]


def _prepare(V, VT, B, w, snd, typ, rcv, cfg):
    NC, NPC, NWIN, CH = cfg.n_cores, cfg.npc, cfg.nwin, cfg.chunks
    E = cfg.n_edges
    snd = np.asarray(snd, np.int64)
    typ = np.asarray(typ, np.int64)
    rcv = np.asarray(rcv, np.int64)
    w = np.asarray(w, np.float32)

    core = rcv // NPC
    rloc = rcv - core * NPC
    win = rloc // cfg.win
    rin = (rloc - win * cfg.win).astype(np.float32)
    chunk = snd // cfg.crows
    sloc = (snd - chunk * cfg.crows).astype(np.int16)

    key = (core * CH + chunk) * NWIN + win
    order = np.lexsort((sloc, key))
    cnt = np.bincount(key, minlength=NC * CH * NWIN).reshape(NC, CH, NWIN)
    S = ((cnt.max(axis=0) + 127) // 128 * 128).astype(np.int64)  # [CH, NWIN]
    offs = np.concatenate([[0], np.cumsum(S.ravel())])[:-1].reshape(CH, NWIN)
    L = int(S.sum())

    cnt_flat = cnt.ravel()
    grp_start = np.concatenate([[0], np.cumsum(cnt_flat)])[:-1]
    ranks = np.arange(E) - np.repeat(grp_start, cnt_flat)
    ids = key[order]
    c_of = (ids // NWIN) % CH
    w_of = ids % NWIN
    slot = offs[c_of, w_of] + ranks
    core_s = core[order]

    sl_s = np.zeros((NC, L), np.int16)
    sl_t = np.zeros((NC, L), np.int16)
    sl_r = np.zeros((NC, L), np.float32)
    sl_w = np.zeros((NC, L), np.float32)
    sl_s[core_s, slot] = sloc[order]
    sl_t[core_s, slot] = typ[order].astype(np.int16)
    sl_r[core_s, slot] = rin[order]
    sl_w[core_s, slot] = w[order]

    iota = np.ascontiguousarray(
        np.tile(np.arange(cfg.win, dtype=np.float16), (128, 1)))
    ident = np.eye(128, dtype=np.float16)
    has_bias = bool(np.any(np.asarray(B) != 0))
    sidx_w = _wrap16(sl_s)
    tidx_w = _wrap16(sl_t)
    recvf = _wrap128(sl_r)
    wf = _wrap128(sl_w)
    VTt = np.ascontiguousarray(np.asarray(V, np.float32).T)       # [128, N]
    VTypt = np.ascontiguousarray(np.asarray(VT, np.float32).T)    # [128, 401]
    in_maps = []
    for i in range(NC):
        m = {"vtabt": VTt, "vtypt": VTypt, "sidx": sidx_w[i],
             "tidx": tidx_w[i], "recvf": recvf[i], "wf": wf[i],
             "iota": iota, "ident": ident}
        if has_bias:
            m["biasd"] = np.ascontiguousarray(
                np.asarray(B, np.float32).reshape(128, 1))
        in_maps.append(m)
    return S, L, has_bias, in_maps


def _run(V, VT, B, w, snd, typ, rcv, cfg=None, trace=False):
    from concourse.bass_utils import run_bass_kernel_spmd
    cfg = cfg or CFG
    S, L, has_bias, in_maps = _prepare(V, VT, B, w, snd, typ, rcv, cfg)
    nc = _get_program(S, L, has_bias, cfg)
    res = run_bass_kernel_spmd(nc, in_maps, list(range(cfg.n_cores)),
                               trace=trace)
    out = np.concatenate(
        [np.asarray(res.results[i]["out"]).astype(np.float32)[:, :cfg.npc].T
         for i in range(cfg.n_cores)], 0)
    return np.ascontiguousarray(out[:cfg.n_nodes]), res


def kernel(V_proj_sender, V_types, B_message, inc_weights,
           sender_idx, type_idx, recv_idx):
    out, _ = _run(V_proj_sender, V_types, B_message, inc_weights,
                  sender_idx, type_idx, recv_idx)
    return out


# revision 43
# speedup vs baseline: 1.1255x; 1.0904x over previous
"""DiagGCN message-passing kernel for 8 Trainium2 NeuronCores (V3).

V3 = V2 with the TYPE gather moved off the Pool engine onto PE:
  - Pool does ONE ap_gather per span (senders only, ~26.6 ns/idx on HW),
    halving the Pool roofline to ~2.1 ms.
  - Types are gathered by PE as one-hot matmuls against the SBUF-resident
    fp16 type table [t, d] (4 x 128 tiles): per quarter-span (2 tiles),
    DVE builds [e, 512] type one-hots, PE transposes them into a PSUM
    bank (fp16), ACT copies them to SBUF, and PE contracts
    psum_g[d, e] += table_tile[t, d]^T @ ohT[t, e]. DVE then multiplies
    psum_g by the gathered sender columns into mrel [d, e] fp16 — the
    rest of the pipeline (relu, msg transpose, recv one-hot scatter,
    drains) is unchanged from V2.
  - PSUM: windows (1024 nodes = 2 banks) rotate over pairs {0,1},{2,3};
    type path double-buffers banks 4,5 per quarter; msg transposes use
    banks 6,7.
"""
import sys
sys.path.insert(0, "/opt/trn_rl_repo")
import numpy as np
from dataclasses import dataclass


@dataclass(frozen=True)
class Config:
    n_nodes: int = 100000
    n_edges: int = 600000
    d: int = 128
    n_types: int = 401
    n_cores: int = 8
    chunks: int = 4
    span: int = 1024
    win: int = 1024

    @property
    def npc(self):
        return self.n_nodes // self.n_cores

    @property
    def nwin(self):
        return (self.npc + self.win - 1) // self.win

    @property
    def crows(self):
        return (self.n_nodes + self.chunks - 1) // self.chunks


CFG = Config()

_PROGRAM_CACHE = {}


def _wrap16(arr):
    nc_, L = arr.shape
    a = arr.astype(np.int16).reshape(nc_, L // 16, 16).transpose(0, 2, 1)
    return np.ascontiguousarray(np.tile(a, (1, 8, 1)))


def _wrap128(arr, dtype=np.float32):
    nc_, L = arr.shape
    a = arr.astype(dtype).reshape(nc_, L // 128, 128).transpose(0, 2, 1)
    return np.ascontiguousarray(a)


def _schedule(S, cfg):
    chunks, nwin = S.shape
    offs = np.concatenate([[0], np.cumsum(S.ravel())])[:-1].reshape(chunks, nwin)
    L = int(S.sum())
    first_chunk = np.full(nwin, -1, np.int64)
    for c in range(chunks):
        m = (S[c] > 0) & (first_chunk < 0)
        first_chunk[m] = c
    memset_windows = [w for w in range(nwin) if first_chunk[w] < 0]

    spans = []
    gw = -1
    windows = []
    for c in range(chunks):
        Lc = int(S[c].sum())
        if Lc == 0:
            continue
        c_off = int(offs[c, 0])
        tiles = []
        for w in range(nwin):
            nt = int(S[c, w]) // 128
            if nt == 0:
                continue
            gw += 1
            windows.append(dict(c=c, w=w, gw=gw,
                                mode="copy" if first_chunk[w] == c else "add"))
            for k in range(nt):
                tiles.append(dict(w=w, gw=gw, first=(k == 0), last=(k == nt - 1)))
        pos = 0
        while pos < Lc:
            n = min(cfg.span, Lc - pos)
            t0 = pos // 128
            spans.append(dict(off=c_off + pos, n=n, chunk=c,
                              tiles=tiles[t0:t0 + n // 128]))
            pos += n
    for sp in spans:
        sp["ending"] = [t["gw"] for t in sp["tiles"] if t["last"]]
    # quarters: per span, groups of up to 2 tiles; global index j
    j = 0
    for sp in spans:
        nt = sp["n"] // 128
        qs = []
        for q in range((nt + 1) // 2):
            qs.append(dict(j=j, tis=list(range(2 * q, min(2 * q + 2, nt)))))
            j += 1
        sp["quarters"] = qs
    chunk_first_span = {}
    chunk_last_span = {}
    for s, sp in enumerate(spans):
        c = sp["chunk"]
        if c not in chunk_first_span:
            chunk_first_span[c] = s
        chunk_last_span[c] = s
    return dict(spans=spans, windows=windows, memset_windows=memset_windows,
                L=L, offs=offs, n_windows=gw + 1, n_quarters=j,
                chunk_first_span=chunk_first_span,
                chunk_last_span=chunk_last_span)


def _build_program(S_bytes, L, has_bias, cfg):
    import concourse.bacc as bacc
    import concourse.mybir as mybir
    from concourse.library_config import ap_gather as ap_gather_lib

    S = np.frombuffer(S_bytes, np.int64).reshape(cfg.chunks, cfg.nwin)
    sch = _schedule(S, cfg)
    spans, windows = sch["spans"], sch["windows"]
    nspan = len(spans)
    n_windows = sch["n_windows"]
    NWIN, D, WIN = cfg.nwin, cfg.d, cfg.win
    CROWS = cfg.crows
    f32 = mybir.dt.float32
    f16 = mybir.dt.float16
    chunks_used = sorted(sch["chunk_first_span"].keys())
    last_span_of = sch["chunk_last_span"]

    nc = bacc.Bacc("TRN2", debug=True)
    vtabt = nc.dram_tensor("vtabt", [D, cfg.n_nodes], f32, kind="ExternalInput")
    vtypd_d = nc.dram_tensor("vtypd", [128, 4, 128], f16, kind="ExternalInput")
    sidx_d = nc.dram_tensor("sidx", [128, L // 16], mybir.dt.int16, kind="ExternalInput")
    tf_d = nc.dram_tensor("tf", [128, L // 128], f32, kind="ExternalInput")
    recvf_d = nc.dram_tensor("recvf", [128, L // 128], f32, kind="ExternalInput")
    wf_d = nc.dram_tensor("wf", [128, L // 128], f32, kind="ExternalInput")
    iota_d = nc.dram_tensor("iota", [128, WIN], f16, kind="ExternalInput")
    ident_d = nc.dram_tensor("ident", [128, 128], f16, kind="ExternalInput")
    if has_bias:
        bias_d = nc.dram_tensor("biasd", [128, 1], f32, kind="ExternalInput")
    out_d = nc.dram_tensor("out", [128, NWIN * WIN], f16, kind="ExternalOutput")

    from contextlib import ExitStack
    with ExitStack() as ctx:
        tb = ctx.enter_context(nc.sbuf_tensor("tb0", [128, CROWS], f32))
        vtyp_pd = ctx.enter_context(nc.sbuf_tensor("vtyp_pd", [128, 4, 128], f16))
        sidx_t = ctx.enter_context(nc.sbuf_tensor("sidx_t", [128, L // 16], mybir.dt.int16))
        tf_t = ctx.enter_context(nc.sbuf_tensor("tf_t", [128, L // 128], f32))
        recvf_t = ctx.enter_context(nc.sbuf_tensor("recvf_t", [128, L // 128], f32))
        wf_t = ctx.enter_context(nc.sbuf_tensor("wf_t", [128, L // 128], f32))
        iota_t = ctx.enter_context(nc.sbuf_tensor("iota_t", [128, WIN], f16))
        ident_t = ctx.enter_context(nc.sbuf_tensor("ident_t", [128, 128], f16))
        if has_bias:
            bias_t = ctx.enter_context(nc.sbuf_tensor("bias_t", [128, 1], f32))
        sgT = [ctx.enter_context(nc.sbuf_tensor(f"sgT{i}", [128, cfg.span], f32))
               for i in range(2)]
        NM = 2
        mrel = [ctx.enter_context(nc.sbuf_tensor(f"mrel{i}", [128, cfg.span], f16))
                for i in range(NM)]
        NR = 2
        rspan = [ctx.enter_context(nc.sbuf_tensor(f"rspan{i}", [128, cfg.span], f16))
                 for i in range(NR)]
        NO = 2
        ohb = [ctx.enter_context(nc.sbuf_tensor(f"ohb{i}", [128, 8, WIN], f16))
               for i in range(NO)]
        # type-path quarter buffers
        ohtb = [ctx.enter_context(nc.sbuf_tensor(f"ohtb{i}", [128, 2, 512], f16))
                for i in range(2)]
        tohs = [ctx.enter_context(nc.sbuf_tensor(f"tohs{i}", [128, 8, 128], f16))
                for i in range(2)]
        accum = ctx.enter_context(nc.sbuf_tensor("accum", [128, NWIN * WIN], f16))
        psum = ctx.enter_context(nc.psum_tensor("psum", [128, 8, 512], f32))

        ld = ctx.enter_context(nc.semaphore("ld"))
        ldc = ctx.enter_context(nc.semaphore("ldc"))
        gs = ctx.enter_context(nc.semaphore("gs"))
        vm = ctx.enter_context(nc.semaphore("vm"))
        ar = ctx.enter_context(nc.semaphore("ar"))
        pt = ctx.enter_context(nc.semaphore("pt"))
        ac = ctx.enter_context(nc.semaphore("ac"))
        ohs = ctx.enter_context(nc.semaphore("ohs"))
        mm = ctx.enter_context(nc.semaphore("mm"))
        rs = ctx.enter_context(nc.semaphore("rs"))
        pes = ctx.enter_context(nc.semaphore("pes"))
        od = ctx.enter_context(nc.semaphore("od"))
        fin = ctx.enter_context(nc.semaphore("fin"))
        # type-path sems (per global quarter j)
        ohts = ctx.enter_context(nc.semaphore("ohts"))
        tt = ctx.enter_context(nc.semaphore("tt"))
        tc = ctx.enter_context(nc.semaphore("tc"))
        tg = ctx.enter_context(nc.semaphore("tg"))
        mq = ctx.enter_context(nc.semaphore("mq"))
        block = ctx.enter_context(nc.Block())

        n_aux = 7 if has_bias else 6

        # ---- SP: all bulk DMA ----
        @block.sync
        def _(sp):
            sp.dma_start(sidx_t[:], sidx_d[:]).then_inc(ld, 16)
            sp.dma_start(tf_t[:], tf_d[:]).then_inc(ld, 16)
            sp.dma_start(recvf_t[:], recvf_d[:]).then_inc(ld, 16)
            sp.dma_start(wf_t[:], wf_d[:]).then_inc(ld, 16)
            sp.dma_start(iota_t[:], iota_d[:]).then_inc(ld, 16)
            sp.dma_start(ident_t[:], ident_d[:]).then_inc(ld, 16)
            if has_bias:
                sp.dma_start(bias_t[:], bias_d[:]).then_inc(ld, 16)
            sp.dma_start(vtyp_pd[:], vtypd_d[:]).then_inc(ldc, 16)
            for i, c in enumerate(chunks_used):
                if i >= 1:
                    prev_last = last_span_of[chunks_used[i - 1]]
                    sp.wait_ge(gs, prev_last + 1)
                c0 = c * CROWS
                crn = min(CROWS, cfg.n_nodes - c0)
                sp.dma_start(tb[:, :crn], vtabt[:, c0:c0 + crn]
                             ).then_inc(ldc, 16)
            sp.wait_ge(rs, n_windows)
            sp.wait_ge(fin, 1)
            sp.dma_start(out_d[:], accum[:]).then_inc(od, 16)
            sp.wait_ge(od, 16)

        # ---- Pool: sender gathers only ----
        @block.gpsimd
        def _(g):
            g.load_library(ap_gather_lib)
            g.wait_ge(ld, 16)   # sidx loaded
            for s, sp_ in enumerate(spans):
                k, n, off = s % 2, sp_["n"], sp_["off"]
                ci = chunks_used.index(sp_["chunk"])
                g.wait_ge(ldc, 16 * (ci + 2))
                if s >= 2:
                    g.wait_ge(vm, s - 1)   # sgT[k] free (muls s-2 done)
                crn = min(CROWS, cfg.n_nodes - sp_["chunk"] * CROWS)
                g.ap_gather(sgT[k][:, :n], tb[:, :crn],
                            sidx_t[:, off // 16:(off + n) // 16],
                            128, crn, 1, n).then_inc(gs, 1)

        # ---- DVE: type one-hots, recv one-hots, drains, muls ----
        @block.vector
        def _(v):
            v.wait_ge(ld, 16 * n_aux)
            for w in sch["memset_windows"]:
                v.memset(accum[:, w * WIN:(w + 1) * WIN], 0.0)

            def drain_window(wi):
                win = windows[wi]
                v.wait_ge(mm, wi + 1)
                dst = accum[:, win["w"] * WIN:(win["w"] + 1) * WIN]
                b = 2 * (wi % 2)
                src = psum[:, b:b + 2, :].rearrange("p a b -> p (a b)")
                if win["mode"] == "copy":
                    v.tensor_copy(dst, src)
                else:
                    v.tensor_add(dst, dst, src)
                v.sem_inc(rs, 1)

            iota512 = iota_t[:, :512].rearrange("p (o d) -> p o d", o=1)
            iotaW = iota_t[:, :].rearrange("p (o d) -> p o d", o=1)
            for s, sp_ in enumerate(spans):
                k, n, off = s % 2, sp_["n"], sp_["off"]
                ko = s % NO
                nt = n // 128
                # type one-hots per quarter
                for qd in sp_["quarters"]:
                    j = qd["j"]
                    if j >= 2:
                        v.wait_ge(tt, j - 1)     # ohtb[j%2] free
                    for ii, ti in enumerate(qd["tis"]):
                        col = off // 128 + ti
                        v.tensor_scalar(
                            ohtb[j % 2][:, ii, :], iota512,
                            tf_t[:, col:col + 1], 1.0,
                            mybir.AluOpType.is_equal,
                            mybir.AluOpType.mult,
                        )
                    v.drain().then_inc(ohts, 1)
                # recv one-hots
                if s >= NO:
                    v.wait_ge(pes, s - NO + 1)
                for i in range(nt):
                    col = off // 128 + i
                    v.tensor_scalar(
                        ohb[ko][:, i, :], iotaW,
                        recvf_t[:, col:col + 1],
                        wf_t[:, col:col + 1],
                        mybir.AluOpType.is_equal,
                        mybir.AluOpType.mult,
                    )
                v.drain().then_inc(ohs, 1)
                # drains two spans back
                if s >= 2:
                    for wi in spans[s - 2]["ending"]:
                        drain_window(wi)
                # muls per quarter: mrel[d, e-tile] = psum_g * sgT
                v.wait_ge(gs, s + 1)
                if s >= NM:
                    v.wait_ge(pt, s - NM + 1)
                for qd in sp_["quarters"]:
                    j = qd["j"]
                    v.wait_ge(tg, j + 1)
                    for ii, ti in enumerate(qd["tis"]):
                        v.tensor_mul(
                            mrel[s % NM][:, ti * 128:(ti + 1) * 128],
                            psum[:, 4 + j % 2, ii * 128:(ii + 1) * 128],
                            sgT[k][:, ti * 128:(ti + 1) * 128])
                    v.drain().then_inc(mq, 1)
                v.drain().then_inc(vm, 1)
            for sp_ in spans[-2:]:
                for wi in sp_["ending"]:
                    drain_window(wi)
            v.drain().then_inc(fin, 1)

        # ---- ACT: toh copies, relu, msg-transpose copies ----
        @block.scalar
        def _(a):
            if has_bias:
                a.wait_ge(ld, 16 * n_aux)

            def copy_span(m):
                nn = spans[m]["n"]
                if m >= NR:
                    a.wait_ge(pes, m - NR + 1)
                a.wait_ge(pt, m + 1)
                a.activation(rspan[m % NR][:, :nn],
                             psum[:, 6 + m % 2, :nn // 2].bitcast(f16),
                             mybir.ActivationFunctionType.Copy)
                a.drain().then_inc(ac, 1)

            for s, sp_ in enumerate(spans):
                n = sp_["n"]
                km = s % NM
                for qd in sp_["quarters"]:
                    j = qd["j"]
                    a.wait_ge(tt, j + 1)
                    if j >= 2:
                        a.wait_ge(tg, j - 1)   # tohs[j%2] free
                    nblk = len(qd["tis"]) * 4
                    a.activation(
                        tohs[j % 2][:, :nblk, :],
                        psum[:, 4 + j % 2, :nblk * 64].bitcast(f16),
                        mybir.ActivationFunctionType.Copy)
                    a.drain().then_inc(tc, 1)
                a.wait_ge(vm, s + 1)
                a.activation(mrel[km][:, :n], mrel[km][:, :n],
                             mybir.ActivationFunctionType.Relu,
                             bias=bias_t[:, :] if has_bias else 0.0)
                a.drain().then_inc(ar, 1)
                if s >= 1:
                    copy_span(s - 1)
            copy_span(nspan - 1)

        # ---- PE: type-path transposes+matmuls, msg transposes, scatters ----
        @block.tensor
        def _(t):
            t.wait_ge(ld, 16 * 6)

            def scatter_span(m):
                kr, ko = m % NR, m % NO
                t.wait_ge(ac, m + 1)
                t.wait_ge(ohs, m + 1)
                for i, tile in enumerate(spans[m]["tiles"]):
                    gw = tile["gw"]
                    if tile["first"] and gw >= 2:
                        t.wait_ge(rs, gw - 1)
                    b = 2 * (gw % 2)
                    for h in range(2):
                        inst = t.matmul(
                            psum[:, b + h, :],
                            rspan[kr][:, i * 128:(i + 1) * 128],
                            ohb[ko][:, i, h * 512:(h + 1) * 512],
                            start=tile["first"], stop=tile["last"],
                        )
                    if tile["last"]:
                        inst.then_inc(mm, 1)
                t.drain().then_inc(pes, 1)

            def tohT(qd):
                j = qd["j"]
                t.wait_ge(ohts, j + 1)
                if j >= 2:
                    t.wait_ge(mq, j - 1)   # bank 4+j%2 free (muls j-2 done)
                pbq = psum[:, 4 + j % 2, :].bitcast(f16)
                for ii in range(len(qd["tis"])):
                    for bb in range(4):
                        blk = ii * 4 + bb
                        t.transpose(pbq[:, blk * 128:(blk + 1) * 128],
                                    ohtb[j % 2][:, ii, bb * 128:(bb + 1) * 128],
                                    ident_t[:, :])
                t.drain().then_inc(tt, 1)

            def typemm(qd):
                j = qd["j"]
                t.wait_ge(tc, j + 1)
                for ii in range(len(qd["tis"])):
                    for bb in range(4):
                        t.matmul(
                            psum[:, 4 + j % 2, ii * 128:(ii + 1) * 128],
                            vtyp_pd[:, bb, :],
                            tohs[j % 2][:, ii * 4 + bb, :],
                            start=(bb == 0), stop=(bb == 3),
                        )
                t.drain().then_inc(tg, 1)

            for s, sp_ in enumerate(spans):
                n = sp_["n"]
                km = s % NM
                nt = n // 128
                qs = sp_["quarters"]
                done_mm = 0
                for qi in range(len(qs)):
                    tohT(qs[qi])
                    if qi >= 1:
                        typemm(qs[done_mm])
                        done_mm += 1
                while done_mm < len(qs):
                    typemm(qs[done_mm])
                    done_mm += 1
                # msg transposes
                t.wait_ge(ar, s + 1)
                if s >= 2:
                    t.wait_ge(ac, s - 1)
                pb = psum[:, 6 + s % 2, :n // 2].bitcast(f16)
                for i in range(nt):
                    t.transpose(pb[:, i * 128:(i + 1) * 128],
                                mrel[km][:, i * 128:(i + 1) * 128],
                                ident_t[:, :])
                t.drain().then_inc(pt, 1)
                if s >= 1:
                    scatter_span(s - 1)
            scatter_span(nspan - 1)

    nc.compile()
    return nc


def _get_program(S, L, has_bias, cfg):
    key = (S.tobytes(), L, has_bias, cfg)
    if key not in _PROGRAM_CACHE:
        _PROGRAM_CACHE[key] = _build_program(S.tobytes(), L, has_bias, cfg)
    return _PROGRAM_CACHE[key]


def _prepare(V, VT, B, w, snd, typ, rcv, cfg):
    NC, NPC, NWIN, CH = cfg.n_cores, cfg.npc, cfg.nwin, cfg.chunks
    E = cfg.n_edges
    snd = np.asarray(snd, np.int64)
    typ = np.asarray(typ, np.int64)
    rcv = np.asarray(rcv, np.int64)
    w = np.asarray(w, np.float32)

    core = rcv // NPC
    rloc = rcv - core * NPC
    win = rloc // cfg.win
    rin = (rloc - win * cfg.win).astype(np.float32)
    chunk = snd // cfg.crows
    sloc = (snd - chunk * cfg.crows).astype(np.int16)

    key = (core * CH + chunk) * NWIN + win
    order = np.lexsort((sloc, key))
    cnt = np.bincount(key, minlength=NC * CH * NWIN).reshape(NC, CH, NWIN)
    S = ((cnt.max(axis=0) + 127) // 128 * 128).astype(np.int64)
    offs = np.concatenate([[0], np.cumsum(S.ravel())])[:-1].reshape(CH, NWIN)
    L = int(S.sum())

    cnt_flat = cnt.ravel()
    grp_start = np.concatenate([[0], np.cumsum(cnt_flat)])[:-1]
    ranks = np.arange(E) - np.repeat(grp_start, cnt_flat)
    ids = key[order]
    c_of = (ids // NWIN) % CH
    w_of = ids % NWIN
    slot = offs[c_of, w_of] + ranks
    core_s = core[order]

    sl_s = np.zeros((NC, L), np.int16)
    sl_t = np.zeros((NC, L), np.int16)
    sl_r = np.zeros((NC, L), np.float32)
    sl_w = np.zeros((NC, L), np.float32)
    sl_s[core_s, slot] = sloc[order]
    sl_t[core_s, slot] = typ[order].astype(np.int16)
    sl_r[core_s, slot] = rin[order]
    sl_w[core_s, slot] = w[order]

    iota = np.ascontiguousarray(
        np.tile(np.arange(cfg.win, dtype=np.float16), (128, 1)))
    ident = np.eye(128, dtype=np.float16)
    has_bias = bool(np.any(np.asarray(B) != 0))
    sidx_w = _wrap16(sl_s)
    tf = _wrap128(sl_t)            # type idx as f32 scalars
    recvf = _wrap128(sl_r)
    wf = _wrap128(sl_w)
    VTt = np.ascontiguousarray(np.asarray(V, np.float32).T)
    Tp = np.zeros((512, 128), np.float16)
    Tp[:cfg.n_types] = np.asarray(VT, np.float32).astype(np.float16)
    vtypd = np.ascontiguousarray(
        Tp.reshape(4, 128, 128).transpose(1, 0, 2))   # [t_loc, blk, d]
    in_maps = []
    for i in range(NC):
        m = {"vtabt": VTt, "vtypd": vtypd, "sidx": sidx_w[i],
             "tf": tf[i], "recvf": recvf[i], "wf": wf[i],
             "iota": iota, "ident": ident}
        if has_bias:
            m["biasd"] = np.ascontiguousarray(
                np.asarray(B, np.float32).reshape(128, 1))
        in_maps.append(m)
    return S, L, has_bias, in_maps


def _run(V, VT, B, w, snd, typ, rcv, cfg=None, trace=False):
    from concourse.bass_utils import run_bass_kernel_spmd
    cfg = cfg or CFG
    S, L, has_bias, in_maps = _prepare(V, VT, B, w, snd, typ, rcv, cfg)
    nc = _get_program(S, L, has_bias, cfg)
    res = run_bass_kernel_spmd(nc, in_maps, list(range(cfg.n_cores)),
                               trace=trace)
    out = np.concatenate(
        [np.asarray(res.results[i]["out"]).astype(np.float32)[:, :cfg.npc].T
         for i in range(cfg.n_cores)], 0)
    return np.ascontiguousarray(out[:cfg.n_nodes]), res


def kernel(V_proj_sender, V_types, B_message, inc_weights,
           sender_idx, type_idx, recv_idx):
    out, _ = _run(V_proj_sender, V_types, B_message, inc_weights,
                  sender_idx, type_idx, recv_idx)
    return out
